# revision 2
# baseline (speedup 1.0000x reference)
"""GATv2 2-layer GNN on 8 Trainium2 NeuronCores.

Strategy:
- Sort edges (incl. self-loops) by destination; shard destination nodes
  across 8 cores (12544 padded nodes each, 98 blocks of 128 dsts).
- Segment softmax without max-subtraction (logits are small):
    out[n] = sum_e exp(l_e) * xl[src_e] / sum_e exp(l_e)
  accumulated per 128-dst block in PSUM via one-hot matmuls.
- Dense transforms sharded by node + AllGather of the per-node tables.
"""

import numpy as np

import concourse.bass as bass
import concourse.mybir as mybir
import concourse.tile as tile
from concourse import bacc
from concourse.bass_utils import run_bass_kernel_spmd

P = 128
NCORES = 8
NEG_SLOPE = 0.2
F32 = mybir.dt.float32
I32 = mybir.dt.int32

LAST_RESULTS = None  # test harness reads exec_time from here


def _prep_edges(edge_index, N, n_pc, g1):
    """Sort by dst, shard by dst-owner core, pad per 128-dst block to a
    uniform tile count M_T (multiple of g1). Returns meta [NCORES, T, 128, 3]
    int32 (src, dst_local, seg_local_f32bits) and M_T."""
    E = edge_index.shape[1]
    src = np.concatenate([edge_index[0], np.arange(N, dtype=np.int64)]).astype(np.int64)
    dst = np.concatenate([edge_index[1], np.arange(N, dtype=np.int64)]).astype(np.int64)
    order = np.argsort(dst, kind="stable")
    src = src[order].astype(np.int32)
    dst = dst[order].astype(np.int32)

    n_blocks = n_pc // P
    # counts per (core, block)
    blk_of_dst = dst // P  # global block id, 0 .. NCORES*n_blocks-1
    counts = np.bincount(blk_of_dst, minlength=NCORES * n_blocks)
    tiles_per_block = (counts + P - 1) // P
    m_t = int(tiles_per_block.max())
    m_t = ((m_t + g1 - 1) // g1) * g1  # round to multiple of G

    T = n_blocks * m_t
    meta = np.zeros((NCORES, T, P, 3), dtype=np.int32)
    # pad defaults: src=0, dst_local=0, seg=200.0f (no one-hot match)
    pad_seg = np.float32(200.0).view(np.int32)
    meta[:, :, :, 2] = pad_seg

    blk_starts = np.zeros(NCORES * n_blocks + 1, dtype=np.int64)
    np.cumsum(counts, out=blk_starts[1:])
    for c in range(NCORES):
        base = c * n_pc
        for b in range(n_blocks):
            gb = c * n_blocks + b
            s, e = blk_starts[gb], blk_starts[gb + 1]
            cnt = e - s
            t0 = b * m_t
            flat = meta[c, t0 : t0 + m_t].reshape(m_t * P, 3)
            flat[:cnt, 0] = src[s:e]
            flat[:cnt, 1] = dst[s:e] - base
            flat[:cnt, 2] = (dst[s:e] - base - b * P).astype(np.float32).view(np.int32)
    return meta, m_t


def _build(n_pad, m_t, g1, g2, consts, nonzero_bias):
    """Build the SPMD Bass program. `consts` holds numpy arrays inlined
    into the NEFF (weights, att tiles, iota, identity)."""
    n_pc = n_pad // NCORES
    n_blocks = n_pc // P
    T = n_blocks * m_t
    H1, C1 = 8, 16
    D1 = H1 * C1  # 128
    D2 = 16

    nc = bacc.Bacc("TRN2", target_bir_lowering=False, debug=False, num_devices=NCORES)

    xT = nc.dram_tensor("xT", [P, n_pc], F32, kind="ExternalInput")
    meta = nc.dram_tensor("meta", [T, P, 3], I32, kind="ExternalInput")
    out2 = nc.dram_tensor("out2", [n_pc, D2], F32, kind="ExternalOutput")

    xl_own = nc.dram_tensor("xl_own", [n_pc, D1], F32, kind="Internal")
    xr_own = nc.dram_tensor("xr_own", [n_pc, D1], F32, kind="Internal")
    xl_full = nc.dram_tensor("xl_full", [n_pad, D1], F32, kind="Internal", addr_space="Shared")
    xl2_own = nc.dram_tensor("xl2_own", [n_pc, D2], F32, kind="Internal")
    xr2_own = nc.dram_tensor("xr2_own", [n_pc, D2], F32, kind="Internal")
    xl2_full = nc.dram_tensor("xl2_full", [n_pad, D2], F32, kind="Internal", addr_space="Shared")

    with tile.TileContext(nc) as tc:
        wcat_t = nc.inline_tensor(consts["wcat"], name="wcat")      # [128, 256]
        w2cat_t = nc.inline_tensor(consts["w2cat"], name="w2cat")   # [128, 32]
        att_t = nc.inline_tensor(consts["att_tile"], name="att_tile")    # [128, 128]
        att2_t = nc.inline_tensor(consts["att2_tile"], name="att2_tile")  # [128, 16]
        iota_t = nc.inline_tensor(consts["iota_row"], name="iota_row")    # [128, 128]
        ident_t = nc.inline_tensor(consts["identity"], name="identity")   # [128, 128]

        with tc.tile_pool(name="consts", bufs=1) as cpool:
            wcat_sb = cpool.tile([P, 2 * D1], F32)
            nc.sync.dma_start(out=wcat_sb[:], in_=wcat_t[:])
            w2cat_sb = cpool.tile([P, 2 * D2], F32)
            nc.sync.dma_start(out=w2cat_sb[:], in_=w2cat_t[:])
            att_sb = cpool.tile([P, D1], F32)
            nc.sync.dma_start(out=att_sb[:], in_=att_t[:])
            att2_sb = cpool.tile([P, D2], F32)
            nc.sync.dma_start(out=att2_sb[:], in_=att2_t[:])
            iota_sb = cpool.tile([P, P], F32)
            nc.sync.dma_start(out=iota_sb[:], in_=iota_t[:])
            ident_sb = cpool.tile([P, P], F32)
            nc.sync.dma_start(out=ident_sb[:], in_=ident_t[:])
            if nonzero_bias:
                b1_sb = cpool.tile([P, D1], F32)
                nc.sync.dma_start(out=b1_sb[:], in_=nc.inline_tensor(consts["bias1_tile"], name="bias1_tile")[:])
                b2_sb = cpool.tile([P, D2], F32)
                nc.sync.dma_start(out=b2_sb[:], in_=nc.inline_tensor(consts["bias2_tile"], name="bias2_tile")[:])
                ones_sb = cpool.tile([1, P], F32)
                nc.sync.dma_start(out=ones_sb[:], in_=nc.inline_tensor(consts["ones_row"], name="ones_row")[:])
                b01_t = nc.inline_tensor(consts["bias01_row"], name="bias01_row")  # [1, 256]
                b01_sb = cpool.tile([1, 2 * D1], F32)
                nc.sync.dma_start(out=b01_sb[:], in_=b01_t[:])

            # ---------------- Phase 0: own-node dense transforms ----------------
            with tc.tile_pool(name="p0_sbuf", bufs=3) as p0s, \
                 tc.tile_pool(name="p0_psum", bufs=2, space="PSUM") as p0p:
                for b in range(n_blocks):
                    xt = p0s.tile([P, P], F32, tag="xt")
                    nc.sync.dma_start(out=xt[:], in_=xT[:, b * P : (b + 1) * P])
                    ps = p0p.tile([P, 2 * D1], F32)
                    nc.tensor.matmul(out=ps[:], lhsT=xt[:], rhs=wcat_sb[:],
                                     start=True, stop=not nonzero_bias)
                    if nonzero_bias:
                        nc.tensor.matmul(out=ps[:], lhsT=ones_sb[:], rhs=b01_sb[:],
                                         start=False, stop=True)
                    xl_sb = p0s.tile([P, D1], F32, tag="xl_sb")
                    nc.scalar.copy(out=xl_sb[:], in_=ps[:, 0:D1])
                    nc.sync.dma_start(out=xl_own[b * P : (b + 1) * P, :], in_=xl_sb[:])
                    xr_sb = p0s.tile([P, D1], F32, tag="xr_sb")
                    nc.vector.tensor_copy(out=xr_sb[:], in_=ps[:, D1 : 2 * D1])
                    nc.sync.dma_start(out=xr_own[b * P : (b + 1) * P, :], in_=xr_sb[:])

            nc.gpsimd.collective_compute(
                "AllGather", mybir.AluOpType.bypass,
                replica_groups=[list(range(NCORES))],
                ins=[xl_own[:]], outs=[xl_full[:]])

            # ---------------- Phase 1: layer-1 edge processing ----------------
            n_super1 = m_t // g1
            with tc.tile_pool(name="l1_meta", bufs=3) as mpool, \
                 tc.tile_pool(name="l1_g", bufs=3) as gpool, \
                 tc.tile_pool(name="l1_w", bufs=3) as wpool, \
                 tc.tile_pool(name="l1_rhs", bufs=3) as rpool, \
                 tc.tile_pool(name="l1_psum", bufs=2, space="PSUM") as l1p, \
                 tc.tile_pool(name="l1_flush", bufs=2) as fpool, \
                 tc.tile_pool(name="l1_fps", bufs=2, space="PSUM") as fps:
                for b in range(n_blocks):
                    psum_blk = l1p.tile([P, D1 + H1], F32)
                    for s in range(n_super1):
                        t0 = b * m_t + s * g1
                        mt = mpool.tile([P, g1, 3], I32)
                        nc.sync.dma_start(
                            out=mt[:],
                            in_=meta[t0 : t0 + g1].rearrange("t p c -> p t c"))
                        xl_g = gpool.tile([P, g1, D1], F32, tag="xl_g")
                        xr_g = gpool.tile([P, g1, D1], F32, tag="xr_g")
                        for g in range(g1):
                            nc.gpsimd.indirect_dma_start(
                                out=xl_g[:, g, :], out_offset=None, in_=xl_full[:],
                                in_offset=bass.IndirectOffsetOnAxis(ap=mt[:, g, 0:1], axis=0))
                            nc.gpsimd.indirect_dma_start(
                                out=xr_g[:, g, :], out_offset=None, in_=xr_own[:],
                                in_offset=bass.IndirectOffsetOnAxis(ap=mt[:, g, 1:2], axis=0))
                        # e = xl + xr (leaky applied in place)
                        e = gpool.tile([P, g1, D1], F32, tag="e")
                        nc.vector.tensor_tensor(out=e[:], in0=xl_g[:], in1=xr_g[:],
                                                op=mybir.AluOpType.add)
                        nc.scalar.activation(out=e[:], in_=e[:],
                                             func=mybir.ActivationFunctionType.Prelu,
                                             alpha=NEG_SLOPE)
                        # tmp = leaky * att ; logits = chunk16 reduce
                        tmp = gpool.tile([P, g1, D1], F32, tag="tmp")
                        att_b = bass.AP(tensor=att_sb.tensor, offset=att_sb[:].offset,
                                        ap=[att_sb[:].ap[0], [0, g1], [1, D1]])
                        nc.vector.tensor_tensor(out=tmp[:], in0=e[:], in1=att_b,
                                                op=mybir.AluOpType.mult)
                        lg = wpool.tile([P, g1, H1], F32, tag="lg")
                        nc.vector.tensor_reduce(
                            out=lg[:], in_=tmp[:].rearrange("p g (h c) -> p g h c", h=H1),
                            axis=mybir.AxisListType.X, op=mybir.AluOpType.add)
                        # w = exp(logits) written into rhs tails
                        rhs = rpool.tile([P, g1, D1 + H1], F32, tag="rhs")
                        nc.scalar.activation(out=rhs[:, :, D1 : D1 + H1], in_=lg[:],
                                             func=mybir.ActivationFunctionType.Exp)
                        # msg = xl_g * w_bcast16
                        w_b = bass.AP(tensor=rhs.tensor, offset=rhs[:, :, D1 : D1 + H1].offset,
                                      ap=[rhs[:].ap[0], [D1 + H1, g1], [1, H1], [0, C1]])
                        nc.vector.tensor_tensor(
                            out=rhs[:, :, 0:D1].rearrange("p g (h c) -> p g h c", h=H1),
                            in0=xl_g[:].rearrange("p g (h c) -> p g h c", h=H1),
                            in1=w_b, op=mybir.AluOpType.mult)
                        # one-hots + aggregation matmuls
                        st = rpool.tile([P, g1, P], F32, tag="st")
                        for g in range(g1):
                            nc.vector.tensor_scalar(
                                out=st[:, g, :], in0=iota_sb[:],
                                scalar1=mt[:, g, 2:3].bitcast(F32), scalar2=None,
                                op0=mybir.AluOpType.is_equal)
                            nc.tensor.matmul(out=psum_blk[:], lhsT=st[:, g, :],
                                             rhs=rhs[:, g, :],
                                             start=(s == 0 and g == 0),
                                             stop=(s == n_super1 - 1 and g == g1 - 1))
                    # -------- block flush --------
                    den = fpool.tile([P, H1], F32, tag="den")
                    nc.vector.tensor_scalar(out=den[:], in0=psum_blk[:, D1 : D1 + H1],
                                            scalar1=1e-30, scalar2=None,
                                            op0=mybir.AluOpType.max)
                    rec = fpool.tile([P, H1], F32, tag="rec")
                    nc.vector.reciprocal(out=rec[:], in_=den[:])
                    h1pre = fpool.tile([P, D1], F32, tag="h1pre")
                    rec_b = bass.AP(tensor=rec.tensor, offset=rec[:].offset,
                                    ap=[rec[:].ap[0], [1, H1], [0, C1]])
                    nc.vector.tensor_tensor(
                        out=h1pre[:].rearrange("p (h c) -> p h c", h=H1),
                        in0=psum_blk[:, 0:D1].rearrange("p (h c) -> p h c", h=H1),
                        in1=rec_b, op=mybir.AluOpType.mult)
                    if nonzero_bias:
                        nc.vector.tensor_tensor(out=h1pre[:], in0=h1pre[:], in1=b1_sb[:],
                                                op=mybir.AluOpType.add)
                    # ELU: h1 = exp(min(x,0)) + (max(x,0) - 1)
                    posm1 = fpool.tile([P, D1], F32, tag="posm1")
                    nc.vector.tensor_scalar(out=posm1[:], in0=h1pre[:],
                                            scalar1=0.0, scalar2=-1.0,
                                            op0=mybir.AluOpType.max,
                                            op1=mybir.AluOpType.add)
                    nc.vector.tensor_scalar(out=h1pre[:], in0=h1pre[:], scalar1=0.0,
                                            scalar2=None, op0=mybir.AluOpType.min)
                    nc.scalar.activation(out=h1pre[:], in_=h1pre[:],
                                         func=mybir.ActivationFunctionType.Exp)
                    h1 = fpool.tile([P, D1], F32, tag="h1")
                    nc.vector.tensor_tensor(out=h1[:], in0=h1pre[:], in1=posm1[:],
                                            op=mybir.AluOpType.add)
                    # xlr2 = h1 @ [Wl2|Wr2] via PE transpose
                    h1T_ps = fps.tile([P, P], F32, tag="h1T")
                    nc.tensor.transpose(out=h1T_ps[:], in_=h1[:], identity=ident_sb[:])
                    h1T = fpool.tile([P, P], F32, tag="h1Ts")
                    nc.scalar.copy(out=h1T[:], in_=h1T_ps[:])
                    ps2 = fps.tile([P, 2 * D2], F32, tag="ps2")
                    nc.tensor.matmul(out=ps2[:], lhsT=h1T[:], rhs=w2cat_sb[:],
                                     start=True, stop=True)
                    xlr2_sb = fpool.tile([P, 2 * D2], F32, tag="xlr2")
                    nc.vector.tensor_copy(out=xlr2_sb[:], in_=ps2[:])
                    nc.sync.dma_start(out=xl2_own[b * P : (b + 1) * P, :], in_=xlr2_sb[:, 0:D2])
                    nc.sync.dma_start(out=xr2_own[b * P : (b + 1) * P, :], in_=xlr2_sb[:, D2 : 2 * D2])

            nc.gpsimd.collective_compute(
                "AllGather", mybir.AluOpType.bypass,
                replica_groups=[list(range(NCORES))],
                ins=[xl2_own[:]], outs=[xl2_full[:]])

            # ---------------- Phase 2: layer-2 edge processing ----------------
            n_super2 = m_t // g2
            with tc.tile_pool(name="l2_meta", bufs=3) as mpool2, \
                 tc.tile_pool(name="l2_g", bufs=3) as gpool2, \
                 tc.tile_pool(name="l2_rhs", bufs=3) as rpool2, \
                 tc.tile_pool(name="l2_psum", bufs=2, space="PSUM") as l2p, \
                 tc.tile_pool(name="l2_flush", bufs=2) as fpool2:
                for b in range(n_blocks):
                    psum_blk = l2p.tile([P, D2 + 1], F32)
                    for s in range(n_super2):
                        t0 = b * m_t + s * g2
                        mt = mpool2.tile([P, g2, 3], I32)
                        nc.sync.dma_start(
                            out=mt[:],
                            in_=meta[t0 : t0 + g2].rearrange("t p c -> p t c"))
                        xl_g = gpool2.tile([P, g2, D2], F32, tag="xl2_g")
                        xr_g = gpool2.tile([P, g2, D2], F32, tag="xr2_g")
                        for g in range(g2):
                            nc.gpsimd.indirect_dma_start(
                                out=xl_g[:, g, :], out_offset=None, in_=xl2_full[:],
                                in_offset=bass.IndirectOffsetOnAxis(ap=mt[:, g, 0:1], axis=0))
                            nc.gpsimd.indirect_dma_start(
                                out=xr_g[:, g, :], out_offset=None, in_=xr2_own[:],
                                in_offset=bass.IndirectOffsetOnAxis(ap=mt[:, g, 1:2], axis=0))
                        e = gpool2.tile([P, g2, D2], F32, tag="e2")
                        nc.vector.tensor_tensor(out=e[:], in0=xl_g[:], in1=xr_g[:],
                                                op=mybir.AluOpType.add)
                        nc.scalar.activation(out=e[:], in_=e[:],
                                             func=mybir.ActivationFunctionType.Prelu,
                                             alpha=NEG_SLOPE)
                        tmp = gpool2.tile([P, g2, D2], F32, tag="tmp2")
                        att2_b = bass.AP(tensor=att2_sb.tensor, offset=att2_sb[:].offset,
                                         ap=[att2_sb[:].ap[0], [0, g2], [1, D2]])
                        nc.vector.tensor_tensor(out=tmp[:], in0=e[:], in1=att2_b,
                                                op=mybir.AluOpType.mult)
                        lg = gpool2.tile([P, g2], F32, tag="lg2")
                        nc.vector.tensor_reduce(out=lg[:], in_=tmp[:],
                                                axis=mybir.AxisListType.X,
                                                op=mybir.AluOpType.add)
                        rhs = rpool2.tile([P, g2, D2 + 1], F32, tag="rhs2")
                        nc.scalar.activation(out=rhs[:, :, D2 : D2 + 1], in_=lg[:, :, None],
                                             func=mybir.ActivationFunctionType.Exp)
                        w_b = bass.AP(tensor=rhs.tensor, offset=rhs[:, :, D2 : D2 + 1].offset,
                                      ap=[rhs[:].ap[0], [D2 + 1, g2], [0, D2]])
                        nc.vector.tensor_tensor(out=rhs[:, :, 0:D2], in0=xl_g[:],
                                                in1=w_b, op=mybir.AluOpType.mult)
                        st = rpool2.tile([P, g2, P], F32, tag="st2")
                        for g in range(g2):
                            nc.vector.tensor_scalar(
                                out=st[:, g, :], in0=iota_sb[:],
                                scalar1=mt[:, g, 2:3].bitcast(F32), scalar2=None,
                                op0=mybir.AluOpType.is_equal)
                            nc.tensor.matmul(out=psum_blk[:], lhsT=st[:, g, :],
                                             rhs=rhs[:, g, :],
                                             start=(s == 0 and g == 0),
                                             stop=(s == n_super2 - 1 and g == g2 - 1))
                    # -------- block flush --------
                    den = fpool2.tile([P, 1], F32, tag="den2")
                    nc.vector.tensor_scalar(out=den[:], in0=psum_blk[:, D2 : D2 + 1],
                                            scalar1=1e-30, scalar2=None,
                                            op0=mybir.AluOpType.max)
                    rec = fpool2.tile([P, 1], F32, tag="rec2")
                    nc.vector.reciprocal(out=rec[:], in_=den[:])
                    o2 = fpool2.tile([P, D2], F32, tag="o2")
                    nc.vector.tensor_scalar(out=o2[:], in0=psum_blk[:, 0:D2],
                                            scalar1=rec[:, 0:1], scalar2=None,
                                            op0=mybir.AluOpType.mult)
                    if nonzero_bias:
                        nc.vector.tensor_tensor(out=o2[:], in0=o2[:], in1=b2_sb[:],
                                                op=mybir.AluOpType.add)
                    nc.sync.dma_start(out=out2[b * P : (b + 1) * P, :], in_=o2[:])

    nc.compile()
    return nc


def _install_profile_shim():
    """Provide antenv.axon_hooks (missing on this image) so that
    run_bass_kernel_spmd(trace=True) can capture NTFF profiles."""
    import sys
    import types
    try:
        import antenv.axon_hooks  # noqa: F401
        return
    except ImportError:
        pass
    try:
        from trn_agent_boot.trn_boot import _ntff_profile_via_ctypes
        hook = _ntff_profile_via_ctypes("/opt/axon/libaxon_pjrt.so")
        mod = types.ModuleType("antenv.axon_hooks")
        mod._hook = hook
        mod.get_axon_ntff_profile_hook = lambda: mod._hook
        mod.set_axon_ntff_profile_hook = lambda h: setattr(mod, "_hook", h)
        sys.modules["antenv.axon_hooks"] = mod
    except Exception:
        pass


def kernel(x, edge_index, Wl1, bl1, Wr1, br1, att1, bias1,
           Wl2, bl2, Wr2, br2, att2, bias2, trace=False):
    global LAST_RESULTS
    if trace:
        _install_profile_shim()
    x = np.asarray(x, dtype=np.float32)
    edge_index = np.asarray(edge_index)
    N, F = x.shape
    H1, C1 = np.asarray(att1).shape
    D1 = H1 * C1
    D2 = np.asarray(Wl2).shape[1]

    g1, g2 = 4, 4
    n_pad = ((N + NCORES * P - 1) // (NCORES * P)) * (NCORES * P)
    n_pc = n_pad // NCORES

    meta, m_t = _prep_edges(edge_index, N, n_pc, g1)

    wcat = np.ascontiguousarray(np.concatenate([Wl1, Wr1], axis=1), dtype=np.float32)
    w2cat = np.ascontiguousarray(np.concatenate([Wl2, Wr2], axis=1), dtype=np.float32)
    att_tile = np.tile(np.asarray(att1, np.float32).reshape(1, D1), (P, 1))
    att2_tile = np.tile(np.asarray(att2, np.float32).reshape(1, D2), (P, 1))
    iota_row = np.tile(np.arange(P, dtype=np.float32)[None, :], (P, 1))
    identity = np.eye(P, dtype=np.float32)

    bias01 = np.concatenate([np.asarray(bl1, np.float32), np.asarray(br1, np.float32)])
    nonzero_bias = bool(np.any(bias01) or np.any(bias1) or np.any(bl2) or np.any(br2) or np.any(bias2))
    consts = dict(wcat=wcat, w2cat=w2cat, att_tile=att_tile, att2_tile=att2_tile,
                  iota_row=iota_row, identity=identity)
    if nonzero_bias:
        consts["bias1_tile"] = np.tile(np.asarray(bias1, np.float32).reshape(1, D1), (P, 1))
        consts["bias2_tile"] = np.tile(np.asarray(bias2, np.float32).reshape(1, D2), (P, 1))
        consts["ones_row"] = np.ones((1, P), np.float32)
        consts["bias01_row"] = bias01.reshape(1, 2 * D1)
        # fold the ELU "-1" correction into nothing: handled in-kernel already

    nc = _build(n_pad, m_t, g1, g2, consts, nonzero_bias)

    # per-core inputs
    xpadT = np.zeros((P, n_pad), dtype=np.float32)
    xpadT[:, :N] = x.T
    in_maps = []
    for c in range(NCORES):
        in_maps.append({
            "xT": np.ascontiguousarray(xpadT[:, c * n_pc : (c + 1) * n_pc]),
            "meta": meta[c],
        })

    LAST_RESULTS = run_bass_kernel_spmd(nc, in_maps, core_ids=list(range(NCORES)),
                                        trace=trace)
    out = np.concatenate([r["out2"] for r in LAST_RESULTS.results], axis=0)
    return np.ascontiguousarray(out[:N])


# revision 8
# speedup vs baseline: 1.2173x; 1.2173x over previous
"""GATv2 2-layer GNN on 8 Trainium2 NeuronCores.

Strategy:
- Sort edges (incl. self-loops) by destination; shard destination nodes
  across 8 cores (12544 padded nodes each, 98 blocks of 128 dsts).
- Segment softmax without max-subtraction (logits are small):
    out[n] = sum_e exp(l_e) * xl[src_e] / sum_e exp(l_e)
  accumulated per 128-dst block in PSUM via one-hot matmuls.
- Dense transforms sharded by node + AllGather of the per-node tables.
"""

import numpy as np

import concourse.bass as bass
import concourse.mybir as mybir
import concourse.tile as tile
from concourse import bacc
from concourse.bass_utils import run_bass_kernel_spmd

P = 128
NCORES = 8
NEG_SLOPE = 0.2
F32 = mybir.dt.float32
I32 = mybir.dt.int32

LAST_RESULTS = None  # test harness reads exec_time from here


def _prep_edges(edge_index, N, n_pc, g1):
    """Sort by dst, shard by dst-owner core, pad per 128-dst block to a
    uniform tile count M_T (multiple of g1). Returns meta [NCORES, T, 128, 3]
    int32 (src, dst_local, seg_local_f32bits) and M_T."""
    E = edge_index.shape[1]
    src = np.concatenate([edge_index[0], np.arange(N, dtype=np.int64)]).astype(np.int64)
    dst = np.concatenate([edge_index[1], np.arange(N, dtype=np.int64)]).astype(np.int64)
    order = np.argsort(dst, kind="stable")
    src = src[order].astype(np.int32)
    dst = dst[order].astype(np.int32)

    n_blocks = n_pc // P
    # counts per (core, block)
    blk_of_dst = dst // P  # global block id, 0 .. NCORES*n_blocks-1
    counts = np.bincount(blk_of_dst, minlength=NCORES * n_blocks)
    tiles_per_block = (counts + P - 1) // P
    m_t = int(tiles_per_block.max())
    m_t = ((m_t + g1 - 1) // g1) * g1  # round to multiple of G

    T = n_blocks * m_t
    meta = np.zeros((NCORES, T, P, 3), dtype=np.int32)
    # pad defaults: src=0, dst_local=0, seg=200.0f (no one-hot match)
    pad_seg = np.float32(200.0).view(np.int32)
    meta[:, :, :, 2] = pad_seg

    blk_starts = np.zeros(NCORES * n_blocks + 1, dtype=np.int64)
    np.cumsum(counts, out=blk_starts[1:])
    for c in range(NCORES):
        base = c * n_pc
        for b in range(n_blocks):
            gb = c * n_blocks + b
            s, e = blk_starts[gb], blk_starts[gb + 1]
            cnt = e - s
            t0 = b * m_t
            flat = meta[c, t0 : t0 + m_t].reshape(m_t * P, 3)
            flat[:cnt, 0] = src[s:e]
            flat[:cnt, 1] = dst[s:e] - base
            flat[:cnt, 2] = (dst[s:e] - base - b * P).astype(np.float32).view(np.int32)
    return meta, m_t


def _build(n_pad, m_t, g1, g2, consts, nonzero_bias):
    """Build the SPMD Bass program. `consts` holds numpy arrays inlined
    into the NEFF (weights, att tiles, iota, identity)."""
    n_pc = n_pad // NCORES
    n_blocks = n_pc // P
    T = n_blocks * m_t
    H1, C1 = 8, 16
    D1 = H1 * C1  # 128
    D2 = 16

    nc = bacc.Bacc("TRN2", target_bir_lowering=False, debug=False, num_devices=NCORES)

    xT = nc.dram_tensor("xT", [P, n_pc], F32, kind="ExternalInput")
    meta = nc.dram_tensor("meta", [T, P, 3], I32, kind="ExternalInput")
    segf = nc.dram_tensor("segf", [T, P], F32, kind="ExternalInput")
    out2 = nc.dram_tensor("out2", [n_pc, D2], F32, kind="ExternalOutput")

    xl_own = nc.dram_tensor("xl_own", [n_pc, D1], F32, kind="Internal")
    xr_own = nc.dram_tensor("xr_own", [n_pc, D1], F32, kind="Internal")
    xl_full = nc.dram_tensor("xl_full", [n_pad, D1], F32, kind="Internal", addr_space="Shared")
    xl2_own = nc.dram_tensor("xl2_own", [n_pc, D2], F32, kind="Internal")
    xr2_own = nc.dram_tensor("xr2_own", [n_pc, D2], F32, kind="Internal")
    xl2_full = nc.dram_tensor("xl2_full", [n_pad, D2], F32, kind="Internal", addr_space="Shared")

    with tile.TileContext(nc) as tc:
        wcat_t = nc.inline_tensor(consts["wcat"], name="wcat")      # [128, 256]
        w2cat_t = nc.inline_tensor(consts["w2cat"], name="w2cat")   # [128, 32]
        att_t = nc.inline_tensor(consts["att_tile"], name="att_tile")    # [128, 128]
        att2_t = nc.inline_tensor(consts["att2_tile"], name="att2_tile")  # [128, 16]
        iota_t = nc.inline_tensor(consts["iota_row"], name="iota_row")    # [128, 128]
        ident_t = nc.inline_tensor(consts["identity"], name="identity")   # [128, 128]

        with tc.tile_pool(name="consts", bufs=1) as cpool:
            wcat_sb = cpool.tile([P, 2 * D1], F32)
            nc.sync.dma_start(out=wcat_sb[:], in_=wcat_t[:])
            w2cat_sb = cpool.tile([P, 2 * D2], F32)
            nc.sync.dma_start(out=w2cat_sb[:], in_=w2cat_t[:])
            att_sb = cpool.tile([P, D1], F32)
            nc.sync.dma_start(out=att_sb[:], in_=att_t[:])
            att2_sb = cpool.tile([P, D2], F32)
            nc.sync.dma_start(out=att2_sb[:], in_=att2_t[:])
            iota_sb = cpool.tile([P, P], F32)
            nc.sync.dma_start(out=iota_sb[:], in_=iota_t[:])
            iotac_t = nc.inline_tensor(consts["iota_col"], name="iota_col")  # [128, 1]
            iotac_sb = cpool.tile([P, 1], F32)
            nc.sync.dma_start(out=iotac_sb[:], in_=iotac_t[:])
            ident_sb = cpool.tile([P, P], F32)
            nc.sync.dma_start(out=ident_sb[:], in_=ident_t[:])
            if nonzero_bias:
                b1_sb = cpool.tile([P, D1], F32)
                nc.sync.dma_start(out=b1_sb[:], in_=nc.inline_tensor(consts["bias1_tile"], name="bias1_tile")[:])
                b2_sb = cpool.tile([P, D2], F32)
                nc.sync.dma_start(out=b2_sb[:], in_=nc.inline_tensor(consts["bias2_tile"], name="bias2_tile")[:])
                ones_sb = cpool.tile([1, P], F32)
                nc.sync.dma_start(out=ones_sb[:], in_=nc.inline_tensor(consts["ones_row"], name="ones_row")[:])
                b01_t = nc.inline_tensor(consts["bias01_row"], name="bias01_row")  # [1, 256]
                b01_sb = cpool.tile([1, 2 * D1], F32)
                nc.sync.dma_start(out=b01_sb[:], in_=b01_t[:])

            # ---------------- Phase 0: own-node dense transforms ----------------
            with tc.tile_pool(name="p0_sbuf", bufs=3) as p0s, \
                 tc.tile_pool(name="p0_psum", bufs=2, space="PSUM") as p0p:
                for b in range(n_blocks):
                    xt = p0s.tile([P, P], F32, tag="xt")
                    nc.sync.dma_start(out=xt[:], in_=xT[:, b * P : (b + 1) * P])
                    ps = p0p.tile([P, 2 * D1], F32)
                    nc.tensor.matmul(out=ps[:], lhsT=xt[:], rhs=wcat_sb[:],
                                     start=True, stop=not nonzero_bias)
                    if nonzero_bias:
                        nc.tensor.matmul(out=ps[:], lhsT=ones_sb[:], rhs=b01_sb[:],
                                         start=False, stop=True)
                    xl_sb = p0s.tile([P, D1], F32, tag="xl_sb")
                    nc.scalar.copy(out=xl_sb[:], in_=ps[:, 0:D1])
                    nc.sync.dma_start(out=xl_own[b * P : (b + 1) * P, :], in_=xl_sb[:])
                    xr_sb = p0s.tile([P, D1], F32, tag="xr_sb")
                    nc.vector.tensor_copy(out=xr_sb[:], in_=ps[:, D1 : 2 * D1])
                    nc.sync.dma_start(out=xr_own[b * P : (b + 1) * P, :], in_=xr_sb[:])

            nc.gpsimd.collective_compute(
                "AllGather", mybir.AluOpType.bypass,
                replica_groups=[list(range(NCORES))],
                ins=[xl_own[:]], outs=[xl_full[:]])

            # ---------------- Phase 1: layer-1 edge processing ----------------
            n_super1 = m_t // g1
            with tc.tile_pool(name="l1_meta", bufs=3) as mpool, \
                 tc.tile_pool(name="l1_g", bufs=3) as gpool, \
                 tc.tile_pool(name="l1_w", bufs=3) as wpool, \
                 tc.tile_pool(name="l1_rhs", bufs=3) as rpool, \
                 tc.tile_pool(name="l1_psum", bufs=2, space="PSUM") as l1p, \
                 tc.tile_pool(name="l1_flush", bufs=2) as fpool, \
                 tc.tile_pool(name="l1_fps", bufs=2, space="PSUM") as fps:
                for b in range(n_blocks):
                    psum_blk = l1p.tile([P, D1 + H1], F32)
                    for s in range(n_super1):
                        t0 = b * m_t + s * g1
                        mt = mpool.tile([P, g1, 3], I32)
                        nc.sync.dma_start(
                            out=mt[:],
                            in_=meta[t0 : t0 + g1].rearrange("t p c -> p t c"))
                        xl_g = gpool.tile([P, g1, D1], F32, tag="xl_g")
                        xr_g = gpool.tile([P, g1, D1], F32, tag="xr_g")
                        for g in range(g1):
                            nc.gpsimd.indirect_dma_start(
                                out=xl_g[:, g, :], out_offset=None, in_=xl_full[:],
                                in_offset=bass.IndirectOffsetOnAxis(ap=mt[:, g, 0:1], axis=0))
                            nc.gpsimd.indirect_dma_start(
                                out=xr_g[:, g, :], out_offset=None, in_=xr_own[:],
                                in_offset=bass.IndirectOffsetOnAxis(ap=mt[:, g, 1:2], axis=0))
                        # e = xl + xr (leaky applied in place)
                        e = gpool.tile([P, g1, D1], F32, tag="e")
                        nc.vector.tensor_tensor(out=e[:], in0=xl_g[:], in1=xr_g[:],
                                                op=mybir.AluOpType.add)
                        nc.scalar.activation(out=e[:], in_=e[:],
                                             func=mybir.ActivationFunctionType.Prelu,
                                             alpha=NEG_SLOPE)
                        # tmp = leaky * att ; logits = chunk16 reduce
                        tmp = gpool.tile([P, g1, D1], F32, tag="tmp")
                        att_b = bass.AP(tensor=att_sb.tensor, offset=att_sb[:].offset,
                                        ap=[att_sb[:].ap[0], [0, g1], [1, D1]])
                        nc.vector.tensor_tensor(out=tmp[:], in0=e[:], in1=att_b,
                                                op=mybir.AluOpType.mult)
                        lg = wpool.tile([P, g1, H1], F32, tag="lg")
                        nc.vector.tensor_reduce(
                            out=lg[:], in_=tmp[:].rearrange("p g (h c) -> p g h c", h=H1),
                            axis=mybir.AxisListType.X, op=mybir.AluOpType.add)
                        # w = exp(logits) written into rhs tails
                        rhs = rpool.tile([P, g1, D1 + H1], F32, tag="rhs")
                        nc.scalar.activation(out=rhs[:, :, D1 : D1 + H1], in_=lg[:],
                                             func=mybir.ActivationFunctionType.Exp)
                        # msg = xl_g * w_bcast16
                        w_b = bass.AP(tensor=rhs.tensor, offset=rhs[:, :, D1 : D1 + H1].offset,
                                      ap=[rhs[:].ap[0], [D1 + H1, g1], [1, H1], [0, C1]])
                        nc.vector.tensor_tensor(
                            out=rhs[:, :, 0:D1].rearrange("p g (h c) -> p g h c", h=H1),
                            in0=xl_g[:].rearrange("p g (h c) -> p g h c", h=H1),
                            in1=w_b, op=mybir.AluOpType.mult)
                        # one-hots + aggregation matmuls
                        st = rpool.tile([P, g1, P], F32, tag="st")
                        for g in range(g1):
                            nc.vector.tensor_scalar(
                                out=st[:, g, :], in0=iota_sb[:],
                                scalar1=mt[:, g, 2:3].bitcast(F32), scalar2=None,
                                op0=mybir.AluOpType.is_equal)
                            nc.tensor.matmul(out=psum_blk[:], lhsT=st[:, g, :],
                                             rhs=rhs[:, g, :],
                                             start=(s == 0 and g == 0),
                                             stop=(s == n_super1 - 1 and g == g1 - 1))
                    # -------- block flush --------
                    den = fpool.tile([P, H1], F32, tag="den")
                    nc.vector.tensor_scalar(out=den[:], in0=psum_blk[:, D1 : D1 + H1],
                                            scalar1=1e-30, scalar2=None,
                                            op0=mybir.AluOpType.max)
                    rec = fpool.tile([P, H1], F32, tag="rec")
                    nc.vector.reciprocal(out=rec[:], in_=den[:])
                    h1pre = fpool.tile([P, D1], F32, tag="h1pre")
                    rec_b = bass.AP(tensor=rec.tensor, offset=rec[:].offset,
                                    ap=[rec[:].ap[0], [1, H1], [0, C1]])
                    nc.vector.tensor_tensor(
                        out=h1pre[:].rearrange("p (h c) -> p h c", h=H1),
                        in0=psum_blk[:, 0:D1].rearrange("p (h c) -> p h c", h=H1),
                        in1=rec_b, op=mybir.AluOpType.mult)
                    if nonzero_bias:
                        nc.vector.tensor_tensor(out=h1pre[:], in0=h1pre[:], in1=b1_sb[:],
                                                op=mybir.AluOpType.add)
                    # ELU: h1 = exp(min(x,0)) + (max(x,0) - 1)
                    posm1 = fpool.tile([P, D1], F32, tag="posm1")
                    nc.vector.tensor_scalar(out=posm1[:], in0=h1pre[:],
                                            scalar1=0.0, scalar2=-1.0,
                                            op0=mybir.AluOpType.max,
                                            op1=mybir.AluOpType.add)
                    nc.vector.tensor_scalar(out=h1pre[:], in0=h1pre[:], scalar1=0.0,
                                            scalar2=None, op0=mybir.AluOpType.min)
                    nc.scalar.activation(out=h1pre[:], in_=h1pre[:],
                                         func=mybir.ActivationFunctionType.Exp)
                    h1 = fpool.tile([P, D1], F32, tag="h1")
                    nc.vector.tensor_tensor(out=h1[:], in0=h1pre[:], in1=posm1[:],
                                            op=mybir.AluOpType.add)
                    # xlr2 = h1 @ [Wl2|Wr2] via PE transpose
                    h1T_ps = fps.tile([P, P], F32, tag="h1T")
                    nc.tensor.transpose(out=h1T_ps[:], in_=h1[:], identity=ident_sb[:])
                    h1T = fpool.tile([P, P], F32, tag="h1Ts")
                    nc.scalar.copy(out=h1T[:], in_=h1T_ps[:])
                    ps2 = fps.tile([P, 2 * D2], F32, tag="ps2")
                    nc.tensor.matmul(out=ps2[:], lhsT=h1T[:], rhs=w2cat_sb[:],
                                     start=True, stop=True)
                    xlr2_sb = fpool.tile([P, 2 * D2], F32, tag="xlr2")
                    nc.vector.tensor_copy(out=xlr2_sb[:], in_=ps2[:])
                    nc.sync.dma_start(out=xl2_own[b * P : (b + 1) * P, :], in_=xlr2_sb[:, 0:D2])
                    nc.sync.dma_start(out=xr2_own[b * P : (b + 1) * P, :], in_=xlr2_sb[:, D2 : 2 * D2])

            nc.gpsimd.collective_compute(
                "AllGather", mybir.AluOpType.bypass,
                replica_groups=[list(range(NCORES))],
                ins=[xl2_own[:]], outs=[xl2_full[:]])

            # ---------------- Phase 2: layer-2 edge processing ----------------
            n_super2 = m_t // g2
            with tc.tile_pool(name="l2_meta", bufs=3) as mpool2, \
                 tc.tile_pool(name="l2_g", bufs=3) as gpool2, \
                 tc.tile_pool(name="l2_rhs", bufs=3) as rpool2, \
                 tc.tile_pool(name="l2_blk", bufs=2) as bpool2, \
                 tc.tile_pool(name="l2_psum", bufs=2, space="PSUM") as l2p, \
                 tc.tile_pool(name="l2_xps", bufs=3, space="PSUM") as xps2, \
                 tc.tile_pool(name="l2_flush", bufs=2) as fpool2:
                for b in range(n_blocks):
                    psum_blk = l2p.tile([P, D2 + 1], F32)
                    xr2_blk = bpool2.tile([P, D2], F32, tag="xr2blk")
                    nc.sync.dma_start(out=xr2_blk[:],
                                      in_=xr2_own[b * P : (b + 1) * P, :])
                    for s in range(n_super2):
                        t0 = b * m_t + s * g2
                        mt = mpool2.tile([P, g2, 3], I32)
                        nc.sync.dma_start(
                            out=mt[:],
                            in_=meta[t0 : t0 + g2].rearrange("t p c -> p t c"))
                        xl_g = gpool2.tile([P, g2, D2], F32, tag="xl2_g")
                        e = gpool2.tile([P, g2, D2], F32, tag="e2")
                        for g in range(g2):
                            nc.gpsimd.indirect_dma_start(
                                out=xl_g[:, g, :], out_offset=None, in_=xl2_full[:],
                                in_offset=bass.IndirectOffsetOnAxis(ap=mt[:, g, 0:1], axis=0))
                            # xr2[dst] = one-hot expansion of the resident block
                            # rows (dst-sorted edges): S_seg[s,e] @ xr2_blk
                            srow = segf[t0 + g]
                            sbrow = gpool2.tile([P, P], F32, tag="sbrow")
                            nc.sync.dma_start(
                                out=sbrow[:],
                                in_=bass.AP(tensor=srow.tensor, offset=srow.offset,
                                            ap=[[0, P]] + list(srow.ap)))
                            sseg = gpool2.tile([P, P], F32, tag="sseg")
                            nc.vector.tensor_scalar(
                                out=sseg[:], in0=sbrow[:], scalar1=iotac_sb[:, 0:1],
                                scalar2=None, op0=mybir.AluOpType.is_equal)
                            xrp = xps2.tile([P, D2], F32, tag="xrp")
                            nc.tensor.matmul(out=xrp[:], lhsT=sseg[:], rhs=xr2_blk[:],
                                             start=True, stop=True)
                            nc.vector.tensor_tensor(out=e[:, g, :], in0=xl_g[:, g, :],
                                                    in1=xrp[:],
                                                    op=mybir.AluOpType.add)
                        nc.scalar.activation(out=e[:], in_=e[:],
                                             func=mybir.ActivationFunctionType.Prelu,
                                             alpha=NEG_SLOPE)
                        tmp = gpool2.tile([P, g2, D2], F32, tag="tmp2")
                        att2_b = bass.AP(tensor=att2_sb.tensor, offset=att2_sb[:].offset,
                                         ap=[att2_sb[:].ap[0], [0, g2], [1, D2]])
                        nc.vector.tensor_tensor(out=tmp[:], in0=e[:], in1=att2_b,
                                                op=mybir.AluOpType.mult)
                        lg = gpool2.tile([P, g2], F32, tag="lg2")
                        nc.vector.tensor_reduce(out=lg[:], in_=tmp[:],
                                                axis=mybir.AxisListType.X,
                                                op=mybir.AluOpType.add)
                        rhs = rpool2.tile([P, g2, D2 + 1], F32, tag="rhs2")
                        nc.scalar.activation(out=rhs[:, :, D2 : D2 + 1], in_=lg[:, :, None],
                                             func=mybir.ActivationFunctionType.Exp)
                        w_b = bass.AP(tensor=rhs.tensor, offset=rhs[:, :, D2 : D2 + 1].offset,
                                      ap=[rhs[:].ap[0], [D2 + 1, g2], [0, D2]])
                        nc.vector.tensor_tensor(out=rhs[:, :, 0:D2], in0=xl_g[:],
                                                in1=w_b, op=mybir.AluOpType.mult)
                        st = rpool2.tile([P, g2, P], F32, tag="st2")
                        for g in range(g2):
                            nc.vector.tensor_scalar(
                                out=st[:, g, :], in0=iota_sb[:],
                                scalar1=mt[:, g, 2:3].bitcast(F32), scalar2=None,
                                op0=mybir.AluOpType.is_equal)
                            nc.tensor.matmul(out=psum_blk[:], lhsT=st[:, g, :],
                                             rhs=rhs[:, g, :],
                                             start=(s == 0 and g == 0),
                                             stop=(s == n_super2 - 1 and g == g2 - 1))
                    # -------- block flush --------
                    den = fpool2.tile([P, 1], F32, tag="den2")
                    nc.vector.tensor_scalar(out=den[:], in0=psum_blk[:, D2 : D2 + 1],
                                            scalar1=1e-30, scalar2=None,
                                            op0=mybir.AluOpType.max)
                    rec = fpool2.tile([P, 1], F32, tag="rec2")
                    nc.vector.reciprocal(out=rec[:], in_=den[:])
                    o2 = fpool2.tile([P, D2], F32, tag="o2")
                    nc.vector.tensor_scalar(out=o2[:], in0=psum_blk[:, 0:D2],
                                            scalar1=rec[:, 0:1], scalar2=None,
                                            op0=mybir.AluOpType.mult)
                    if nonzero_bias:
                        nc.vector.tensor_tensor(out=o2[:], in0=o2[:], in1=b2_sb[:],
                                                op=mybir.AluOpType.add)
                    nc.sync.dma_start(out=out2[b * P : (b + 1) * P, :], in_=o2[:])

    nc.compile()
    return nc


def _install_profile_shim():
    """Provide antenv.axon_hooks (missing on this image) so that
    run_bass_kernel_spmd(trace=True) can capture NTFF profiles."""
    import sys
    import types
    try:
        import antenv.axon_hooks  # noqa: F401
        return
    except ImportError:
        pass
    try:
        from trn_agent_boot.trn_boot import _ntff_profile_via_ctypes
        hook = _ntff_profile_via_ctypes("/opt/axon/libaxon_pjrt.so")
        mod = types.ModuleType("antenv.axon_hooks")
        mod._hook = hook
        mod.get_axon_ntff_profile_hook = lambda: mod._hook
        mod.set_axon_ntff_profile_hook = lambda h: setattr(mod, "_hook", h)
        sys.modules["antenv.axon_hooks"] = mod
    except Exception:
        pass


def kernel(x, edge_index, Wl1, bl1, Wr1, br1, att1, bias1,
           Wl2, bl2, Wr2, br2, att2, bias2, trace=False):
    global LAST_RESULTS
    if trace:
        _install_profile_shim()
    x = np.asarray(x, dtype=np.float32)
    edge_index = np.asarray(edge_index)
    N, F = x.shape
    H1, C1 = np.asarray(att1).shape
    D1 = H1 * C1
    D2 = np.asarray(Wl2).shape[1]

    g1, g2 = 4, 4
    n_pad = ((N + NCORES * P - 1) // (NCORES * P)) * (NCORES * P)
    n_pc = n_pad // NCORES

    meta, m_t = _prep_edges(edge_index, N, n_pc, g1)

    wcat = np.ascontiguousarray(np.concatenate([Wl1, Wr1], axis=1), dtype=np.float32)
    w2cat = np.ascontiguousarray(np.concatenate([Wl2, Wr2], axis=1), dtype=np.float32)
    att_tile = np.tile(np.asarray(att1, np.float32).reshape(1, D1), (P, 1))
    att2_tile = np.tile(np.asarray(att2, np.float32).reshape(1, D2), (P, 1))
    iota_row = np.tile(np.arange(P, dtype=np.float32)[None, :], (P, 1))
    iota_col = np.arange(P, dtype=np.float32)[:, None].copy()
    identity = np.eye(P, dtype=np.float32)

    bias01 = np.concatenate([np.asarray(bl1, np.float32), np.asarray(br1, np.float32)])
    nonzero_bias = bool(np.any(bias01) or np.any(bias1) or np.any(bl2) or np.any(br2) or np.any(bias2))
    consts = dict(wcat=wcat, w2cat=w2cat, att_tile=att_tile, att2_tile=att2_tile,
                  iota_row=iota_row, iota_col=iota_col, identity=identity)
    if nonzero_bias:
        consts["bias1_tile"] = np.tile(np.asarray(bias1, np.float32).reshape(1, D1), (P, 1))
        consts["bias2_tile"] = np.tile(np.asarray(bias2, np.float32).reshape(1, D2), (P, 1))
        consts["ones_row"] = np.ones((1, P), np.float32)
        consts["bias01_row"] = bias01.reshape(1, 2 * D1)
        # fold the ELU "-1" correction into nothing: handled in-kernel already

    nc = _build(n_pad, m_t, g1, g2, consts, nonzero_bias)

    # per-core inputs
    xpadT = np.zeros((P, n_pad), dtype=np.float32)
    xpadT[:, :N] = x.T
    in_maps = []
    for c in range(NCORES):
        in_maps.append({
            "xT": np.ascontiguousarray(xpadT[:, c * n_pc : (c + 1) * n_pc]),
            "meta": meta[c],
            "segf": np.ascontiguousarray(meta[c][:, :, 2]).view(np.float32),
        })

    LAST_RESULTS = run_bass_kernel_spmd(nc, in_maps, core_ids=list(range(NCORES)),
                                        trace=trace)
    out = np.concatenate([r["out2"] for r in LAST_RESULTS.results], axis=0)
    return np.ascontiguousarray(out[:N])


# revision 9
# speedup vs baseline: 1.5042x; 1.2357x over previous
"""GATv2 2-layer GNN on 8 Trainium2 NeuronCores.

Strategy:
- Sort edges (incl. self-loops) by destination; shard destination nodes
  across 8 cores (12544 padded nodes each, 98 blocks of 128 dsts).
- Segment softmax without max-subtraction (logits are small):
    out[n] = sum_e exp(l_e) * xl[src_e] / sum_e exp(l_e)
  accumulated per 128-dst block in PSUM via one-hot matmuls.
- Dense transforms sharded by node + AllGather of the per-node tables.
"""

import numpy as np

import concourse.bass as bass
import concourse.mybir as mybir
import concourse.tile as tile
from concourse import bacc
from concourse.bass_utils import run_bass_kernel_spmd

P = 128
NCORES = 8
NEG_SLOPE = 0.2
F32 = mybir.dt.float32
I32 = mybir.dt.int32

LAST_RESULTS = None  # test harness reads exec_time from here


def _prep_edges(edge_index, N, n_pc, g1):
    """Sort by dst, shard by dst-owner core, pad per 128-dst block to a
    uniform tile count M_T (multiple of g1). Returns meta [NCORES, T, 128, 3]
    int32 (src, dst_local, seg_local_f32bits) and M_T."""
    E = edge_index.shape[1]
    src = np.concatenate([edge_index[0], np.arange(N, dtype=np.int64)]).astype(np.int64)
    dst = np.concatenate([edge_index[1], np.arange(N, dtype=np.int64)]).astype(np.int64)
    order = np.argsort(dst, kind="stable")
    src = src[order].astype(np.int32)
    dst = dst[order].astype(np.int32)

    n_blocks = n_pc // P
    # counts per (core, block)
    blk_of_dst = dst // P  # global block id, 0 .. NCORES*n_blocks-1
    counts = np.bincount(blk_of_dst, minlength=NCORES * n_blocks)
    tiles_per_block = (counts + P - 1) // P
    m_t = int(tiles_per_block.max())
    m_t = ((m_t + g1 - 1) // g1) * g1  # round to multiple of G

    T = n_blocks * m_t
    meta = np.zeros((NCORES, T, P, 3), dtype=np.int32)
    # pad defaults: src=0, dst_local=0, seg=200.0f (no one-hot match)
    pad_seg = np.float32(200.0).view(np.int32)
    meta[:, :, :, 2] = pad_seg

    blk_starts = np.zeros(NCORES * n_blocks + 1, dtype=np.int64)
    np.cumsum(counts, out=blk_starts[1:])
    for c in range(NCORES):
        base = c * n_pc
        for b in range(n_blocks):
            gb = c * n_blocks + b
            s, e = blk_starts[gb], blk_starts[gb + 1]
            cnt = e - s
            t0 = b * m_t
            flat = meta[c, t0 : t0 + m_t].reshape(m_t * P, 3)
            flat[:cnt, 0] = src[s:e]
            flat[:cnt, 1] = dst[s:e] - base
            flat[:cnt, 2] = (dst[s:e] - base - b * P).astype(np.float32).view(np.int32)
    return meta, m_t


def _build(n_pad, m_t, g1, g2, consts, nonzero_bias):
    """Build the SPMD Bass program. `consts` holds numpy arrays inlined
    into the NEFF (weights, att tiles, iota, identity)."""
    n_pc = n_pad // NCORES
    n_blocks = n_pc // P
    T = n_blocks * m_t
    H1, C1 = 8, 16
    D1 = H1 * C1  # 128
    D2 = 16

    nc = bacc.Bacc("TRN2", target_bir_lowering=False, debug=False, num_devices=NCORES)

    xT = nc.dram_tensor("xT", [P, n_pc], F32, kind="ExternalInput")
    meta = nc.dram_tensor("meta", [T, P, 3], I32, kind="ExternalInput")
    segf = nc.dram_tensor("segf", [T, P], F32, kind="ExternalInput")
    out2 = nc.dram_tensor("out2", [n_pc, D2], F32, kind="ExternalOutput")

    xl_own = nc.dram_tensor("xl_own", [n_pc, D1], F32, kind="Internal")
    xr_own = nc.dram_tensor("xr_own", [n_pc, D1], F32, kind="Internal")
    xl_full = nc.dram_tensor("xl_full", [n_pad, D1], F32, kind="Internal", addr_space="Shared")
    xl2_own = nc.dram_tensor("xl2_own", [n_pc, D2], F32, kind="Internal")
    xr2_own = nc.dram_tensor("xr2_own", [n_pc, D2], F32, kind="Internal")
    xl2_full = nc.dram_tensor("xl2_full", [n_pad, D2], F32, kind="Internal", addr_space="Shared")

    with tile.TileContext(nc) as tc:
        wcat_t = nc.inline_tensor(consts["wcat"], name="wcat")      # [128, 256]
        w2cat_t = nc.inline_tensor(consts["w2cat"], name="w2cat")   # [128, 32]
        att_t = nc.inline_tensor(consts["att_tile"], name="att_tile")    # [128, 128]
        att2_t = nc.inline_tensor(consts["att2_tile"], name="att2_tile")  # [128, 16]
        iota_t = nc.inline_tensor(consts["iota_row"], name="iota_row")    # [128, 128]
        ident_t = nc.inline_tensor(consts["identity"], name="identity")   # [128, 128]

        with tc.tile_pool(name="consts", bufs=1) as cpool:
            wcat_sb = cpool.tile([P, 2 * D1], F32)
            nc.sync.dma_start(out=wcat_sb[:], in_=wcat_t[:])
            w2cat_sb = cpool.tile([P, 2 * D2], F32)
            nc.sync.dma_start(out=w2cat_sb[:], in_=w2cat_t[:])
            att_sb = cpool.tile([P, D1], F32)
            nc.sync.dma_start(out=att_sb[:], in_=att_t[:])
            att2_sb = cpool.tile([P, D2], F32)
            nc.sync.dma_start(out=att2_sb[:], in_=att2_t[:])
            iota_sb = cpool.tile([P, P], F32)
            nc.sync.dma_start(out=iota_sb[:], in_=iota_t[:])
            iotac_t = nc.inline_tensor(consts["iota_col"], name="iota_col")  # [128, 1]
            iotac_sb = cpool.tile([P, 1], F32)
            nc.sync.dma_start(out=iotac_sb[:], in_=iotac_t[:])
            ident_sb = cpool.tile([P, P], F32)
            nc.sync.dma_start(out=ident_sb[:], in_=ident_t[:])
            if nonzero_bias:
                b1_sb = cpool.tile([P, D1], F32)
                nc.sync.dma_start(out=b1_sb[:], in_=nc.inline_tensor(consts["bias1_tile"], name="bias1_tile")[:])
                b2_sb = cpool.tile([P, D2], F32)
                nc.sync.dma_start(out=b2_sb[:], in_=nc.inline_tensor(consts["bias2_tile"], name="bias2_tile")[:])
                ones_sb = cpool.tile([1, P], F32)
                nc.sync.dma_start(out=ones_sb[:], in_=nc.inline_tensor(consts["ones_row"], name="ones_row")[:])
                b01_t = nc.inline_tensor(consts["bias01_row"], name="bias01_row")  # [1, 256]
                b01_sb = cpool.tile([1, 2 * D1], F32)
                nc.sync.dma_start(out=b01_sb[:], in_=b01_t[:])

            # ---------------- Phase 0: own-node dense transforms ----------------
            with tc.tile_pool(name="p0_sbuf", bufs=3) as p0s, \
                 tc.tile_pool(name="p0_psum", bufs=2, space="PSUM") as p0p:
                for b in range(n_blocks):
                    xt = p0s.tile([P, P], F32, tag="xt")
                    nc.sync.dma_start(out=xt[:], in_=xT[:, b * P : (b + 1) * P])
                    ps = p0p.tile([P, 2 * D1], F32)
                    nc.tensor.matmul(out=ps[:], lhsT=xt[:], rhs=wcat_sb[:],
                                     start=True, stop=not nonzero_bias)
                    if nonzero_bias:
                        nc.tensor.matmul(out=ps[:], lhsT=ones_sb[:], rhs=b01_sb[:],
                                         start=False, stop=True)
                    xl_sb = p0s.tile([P, D1], F32, tag="xl_sb")
                    nc.scalar.copy(out=xl_sb[:], in_=ps[:, 0:D1])
                    nc.sync.dma_start(out=xl_own[b * P : (b + 1) * P, :], in_=xl_sb[:])
                    xr_sb = p0s.tile([P, D1], F32, tag="xr_sb")
                    nc.vector.tensor_copy(out=xr_sb[:], in_=ps[:, D1 : 2 * D1])
                    nc.sync.dma_start(out=xr_own[b * P : (b + 1) * P, :], in_=xr_sb[:])

            nc.gpsimd.collective_compute(
                "AllGather", mybir.AluOpType.bypass,
                replica_groups=[list(range(NCORES))],
                ins=[xl_own[:]], outs=[xl_full[:]])

            # ---------------- Phase 1: layer-1 edge processing ----------------
            n_super1 = m_t // g1
            with tc.tile_pool(name="l1_meta", bufs=3) as mpool, \
                 tc.tile_pool(name="l1_g", bufs=3) as gpool, \
                 tc.tile_pool(name="l1_w", bufs=3) as wpool, \
                 tc.tile_pool(name="l1_rhs", bufs=3) as rpool, \
                 tc.tile_pool(name="l1_psum", bufs=2, space="PSUM") as l1p, \
                 tc.tile_pool(name="l1_blk", bufs=2) as bpool1, \
                 tc.tile_pool(name="l1_xps", bufs=2, space="PSUM") as xps1, \
                 tc.tile_pool(name="l1_flush", bufs=2) as fpool, \
                 tc.tile_pool(name="l1_fps", bufs=2, space="PSUM") as fps:
                for b in range(n_blocks):
                    psum_blk = l1p.tile([P, D1 + H1], F32)
                    xr1_blk = bpool1.tile([P, D1], F32, tag="xr1blk")
                    nc.sync.dma_start(out=xr1_blk[:],
                                      in_=xr_own[b * P : (b + 1) * P, :])
                    for s in range(n_super1):
                        t0 = b * m_t + s * g1
                        mt = mpool.tile([P, g1, 3], I32)
                        nc.sync.dma_start(
                            out=mt[:],
                            in_=meta[t0 : t0 + g1].rearrange("t p c -> p t c"))
                        xl_g = gpool.tile([P, g1, D1], F32, tag="xl_g")
                        e = gpool.tile([P, g1, D1], F32, tag="e")
                        for g in range(g1):
                            nc.gpsimd.indirect_dma_start(
                                out=xl_g[:, g, :], out_offset=None, in_=xl_full[:],
                                in_offset=bass.IndirectOffsetOnAxis(ap=mt[:, g, 0:1], axis=0))
                            srow = segf[t0 + g]
                            sbrow = gpool.tile([P, P], F32, tag="sbrow1")
                            nc.sync.dma_start(
                                out=sbrow[:],
                                in_=bass.AP(tensor=srow.tensor, offset=srow.offset,
                                            ap=[[0, P]] + list(srow.ap)))
                            sseg = gpool.tile([P, P], F32, tag="sseg1")
                            nc.vector.tensor_scalar(
                                out=sseg[:], in0=sbrow[:], scalar1=iotac_sb[:, 0:1],
                                scalar2=None, op0=mybir.AluOpType.is_equal)
                            xrp = xps1.tile([P, D1], F32, tag="xrp1")
                            nc.tensor.matmul(out=xrp[:], lhsT=sseg[:], rhs=xr1_blk[:],
                                             start=True, stop=True)
                            nc.vector.tensor_tensor(out=e[:, g, :], in0=xl_g[:, g, :],
                                                    in1=xrp[:],
                                                    op=mybir.AluOpType.add)
                        nc.scalar.activation(out=e[:], in_=e[:],
                                             func=mybir.ActivationFunctionType.Prelu,
                                             alpha=NEG_SLOPE)
                        # tmp = leaky * att ; logits = chunk16 reduce
                        tmp = gpool.tile([P, g1, D1], F32, tag="tmp")
                        att_b = bass.AP(tensor=att_sb.tensor, offset=att_sb[:].offset,
                                        ap=[att_sb[:].ap[0], [0, g1], [1, D1]])
                        nc.vector.tensor_tensor(out=tmp[:], in0=e[:], in1=att_b,
                                                op=mybir.AluOpType.mult)
                        lg = wpool.tile([P, g1, H1], F32, tag="lg")
                        nc.vector.tensor_reduce(
                            out=lg[:], in_=tmp[:].rearrange("p g (h c) -> p g h c", h=H1),
                            axis=mybir.AxisListType.X, op=mybir.AluOpType.add)
                        # w = exp(logits) written into rhs tails
                        rhs = rpool.tile([P, g1, D1 + H1], F32, tag="rhs")
                        nc.scalar.activation(out=rhs[:, :, D1 : D1 + H1], in_=lg[:],
                                             func=mybir.ActivationFunctionType.Exp)
                        # msg = xl_g * w_bcast16
                        w_b = bass.AP(tensor=rhs.tensor, offset=rhs[:, :, D1 : D1 + H1].offset,
                                      ap=[rhs[:].ap[0], [D1 + H1, g1], [1, H1], [0, C1]])
                        nc.vector.tensor_tensor(
                            out=rhs[:, :, 0:D1].rearrange("p g (h c) -> p g h c", h=H1),
                            in0=xl_g[:].rearrange("p g (h c) -> p g h c", h=H1),
                            in1=w_b, op=mybir.AluOpType.mult)
                        # one-hots + aggregation matmuls
                        st = rpool.tile([P, g1, P], F32, tag="st")
                        for g in range(g1):
                            nc.vector.tensor_scalar(
                                out=st[:, g, :], in0=iota_sb[:],
                                scalar1=mt[:, g, 2:3].bitcast(F32), scalar2=None,
                                op0=mybir.AluOpType.is_equal)
                            nc.tensor.matmul(out=psum_blk[:], lhsT=st[:, g, :],
                                             rhs=rhs[:, g, :],
                                             start=(s == 0 and g == 0),
                                             stop=(s == n_super1 - 1 and g == g1 - 1))
                    # -------- block flush --------
                    den = fpool.tile([P, H1], F32, tag="den")
                    nc.vector.tensor_scalar(out=den[:], in0=psum_blk[:, D1 : D1 + H1],
                                            scalar1=1e-30, scalar2=None,
                                            op0=mybir.AluOpType.max)
                    rec = fpool.tile([P, H1], F32, tag="rec")
                    nc.vector.reciprocal(out=rec[:], in_=den[:])
                    h1pre = fpool.tile([P, D1], F32, tag="h1pre")
                    rec_b = bass.AP(tensor=rec.tensor, offset=rec[:].offset,
                                    ap=[rec[:].ap[0], [1, H1], [0, C1]])
                    nc.vector.tensor_tensor(
                        out=h1pre[:].rearrange("p (h c) -> p h c", h=H1),
                        in0=psum_blk[:, 0:D1].rearrange("p (h c) -> p h c", h=H1),
                        in1=rec_b, op=mybir.AluOpType.mult)
                    if nonzero_bias:
                        nc.vector.tensor_tensor(out=h1pre[:], in0=h1pre[:], in1=b1_sb[:],
                                                op=mybir.AluOpType.add)
                    # ELU: h1 = exp(min(x,0)) + (max(x,0) - 1)
                    posm1 = fpool.tile([P, D1], F32, tag="posm1")
                    nc.vector.tensor_scalar(out=posm1[:], in0=h1pre[:],
                                            scalar1=0.0, scalar2=-1.0,
                                            op0=mybir.AluOpType.max,
                                            op1=mybir.AluOpType.add)
                    nc.vector.tensor_scalar(out=h1pre[:], in0=h1pre[:], scalar1=0.0,
                                            scalar2=None, op0=mybir.AluOpType.min)
                    nc.scalar.activation(out=h1pre[:], in_=h1pre[:],
                                         func=mybir.ActivationFunctionType.Exp)
                    h1 = fpool.tile([P, D1], F32, tag="h1")
                    nc.vector.tensor_tensor(out=h1[:], in0=h1pre[:], in1=posm1[:],
                                            op=mybir.AluOpType.add)
                    # xlr2 = h1 @ [Wl2|Wr2] via PE transpose
                    h1T_ps = fps.tile([P, P], F32, tag="h1T")
                    nc.tensor.transpose(out=h1T_ps[:], in_=h1[:], identity=ident_sb[:])
                    h1T = fpool.tile([P, P], F32, tag="h1Ts")
                    nc.scalar.copy(out=h1T[:], in_=h1T_ps[:])
                    ps2 = fps.tile([P, 2 * D2], F32, tag="ps2")
                    nc.tensor.matmul(out=ps2[:], lhsT=h1T[:], rhs=w2cat_sb[:],
                                     start=True, stop=True)
                    xlr2_sb = fpool.tile([P, 2 * D2], F32, tag="xlr2")
                    nc.vector.tensor_copy(out=xlr2_sb[:], in_=ps2[:])
                    nc.sync.dma_start(out=xl2_own[b * P : (b + 1) * P, :], in_=xlr2_sb[:, 0:D2])
                    nc.sync.dma_start(out=xr2_own[b * P : (b + 1) * P, :], in_=xlr2_sb[:, D2 : 2 * D2])

            nc.gpsimd.collective_compute(
                "AllGather", mybir.AluOpType.bypass,
                replica_groups=[list(range(NCORES))],
                ins=[xl2_own[:]], outs=[xl2_full[:]])

            # ---------------- Phase 2: layer-2 edge processing ----------------
            n_super2 = m_t // g2
            with tc.tile_pool(name="l2_meta", bufs=3) as mpool2, \
                 tc.tile_pool(name="l2_g", bufs=3) as gpool2, \
                 tc.tile_pool(name="l2_rhs", bufs=3) as rpool2, \
                 tc.tile_pool(name="l2_blk", bufs=2) as bpool2, \
                 tc.tile_pool(name="l2_psum", bufs=2, space="PSUM") as l2p, \
                 tc.tile_pool(name="l2_xps", bufs=3, space="PSUM") as xps2, \
                 tc.tile_pool(name="l2_flush", bufs=2) as fpool2:
                for b in range(n_blocks):
                    psum_blk = l2p.tile([P, D2 + 1], F32)
                    xr2_blk = bpool2.tile([P, D2], F32, tag="xr2blk")
                    nc.sync.dma_start(out=xr2_blk[:],
                                      in_=xr2_own[b * P : (b + 1) * P, :])
                    for s in range(n_super2):
                        t0 = b * m_t + s * g2
                        mt = mpool2.tile([P, g2, 3], I32)
                        nc.sync.dma_start(
                            out=mt[:],
                            in_=meta[t0 : t0 + g2].rearrange("t p c -> p t c"))
                        xl_g = gpool2.tile([P, g2, D2], F32, tag="xl2_g")
                        e = gpool2.tile([P, g2, D2], F32, tag="e2")
                        for g in range(g2):
                            nc.gpsimd.indirect_dma_start(
                                out=xl_g[:, g, :], out_offset=None, in_=xl2_full[:],
                                in_offset=bass.IndirectOffsetOnAxis(ap=mt[:, g, 0:1], axis=0))
                            # xr2[dst] = one-hot expansion of the resident block
                            # rows (dst-sorted edges): S_seg[s,e] @ xr2_blk
                            srow = segf[t0 + g]
                            sbrow = gpool2.tile([P, P], F32, tag="sbrow")
                            nc.sync.dma_start(
                                out=sbrow[:],
                                in_=bass.AP(tensor=srow.tensor, offset=srow.offset,
                                            ap=[[0, P]] + list(srow.ap)))
                            sseg = gpool2.tile([P, P], F32, tag="sseg")
                            nc.vector.tensor_scalar(
                                out=sseg[:], in0=sbrow[:], scalar1=iotac_sb[:, 0:1],
                                scalar2=None, op0=mybir.AluOpType.is_equal)
                            xrp = xps2.tile([P, D2], F32, tag="xrp")
                            nc.tensor.matmul(out=xrp[:], lhsT=sseg[:], rhs=xr2_blk[:],
                                             start=True, stop=True)
                            nc.vector.tensor_tensor(out=e[:, g, :], in0=xl_g[:, g, :],
                                                    in1=xrp[:],
                                                    op=mybir.AluOpType.add)
                        nc.scalar.activation(out=e[:], in_=e[:],
                                             func=mybir.ActivationFunctionType.Prelu,
                                             alpha=NEG_SLOPE)
                        tmp = gpool2.tile([P, g2, D2], F32, tag="tmp2")
                        att2_b = bass.AP(tensor=att2_sb.tensor, offset=att2_sb[:].offset,
                                         ap=[att2_sb[:].ap[0], [0, g2], [1, D2]])
                        nc.vector.tensor_tensor(out=tmp[:], in0=e[:], in1=att2_b,
                                                op=mybir.AluOpType.mult)
                        lg = gpool2.tile([P, g2], F32, tag="lg2")
                        nc.vector.tensor_reduce(out=lg[:], in_=tmp[:],
                                                axis=mybir.AxisListType.X,
                                                op=mybir.AluOpType.add)
                        rhs = rpool2.tile([P, g2, D2 + 1], F32, tag="rhs2")
                        nc.scalar.activation(out=rhs[:, :, D2 : D2 + 1], in_=lg[:, :, None],
                                             func=mybir.ActivationFunctionType.Exp)
                        w_b = bass.AP(tensor=rhs.tensor, offset=rhs[:, :, D2 : D2 + 1].offset,
                                      ap=[rhs[:].ap[0], [D2 + 1, g2], [0, D2]])
                        nc.vector.tensor_tensor(out=rhs[:, :, 0:D2], in0=xl_g[:],
                                                in1=w_b, op=mybir.AluOpType.mult)
                        st = rpool2.tile([P, g2, P], F32, tag="st2")
                        for g in range(g2):
                            nc.vector.tensor_scalar(
                                out=st[:, g, :], in0=iota_sb[:],
                                scalar1=mt[:, g, 2:3].bitcast(F32), scalar2=None,
                                op0=mybir.AluOpType.is_equal)
                            nc.tensor.matmul(out=psum_blk[:], lhsT=st[:, g, :],
                                             rhs=rhs[:, g, :],
                                             start=(s == 0 and g == 0),
                                             stop=(s == n_super2 - 1 and g == g2 - 1))
                    # -------- block flush --------
                    den = fpool2.tile([P, 1], F32, tag="den2")
                    nc.vector.tensor_scalar(out=den[:], in0=psum_blk[:, D2 : D2 + 1],
                                            scalar1=1e-30, scalar2=None,
                                            op0=mybir.AluOpType.max)
                    rec = fpool2.tile([P, 1], F32, tag="rec2")
                    nc.vector.reciprocal(out=rec[:], in_=den[:])
                    o2 = fpool2.tile([P, D2], F32, tag="o2")
                    nc.vector.tensor_scalar(out=o2[:], in0=psum_blk[:, 0:D2],
                                            scalar1=rec[:, 0:1], scalar2=None,
                                            op0=mybir.AluOpType.mult)
                    if nonzero_bias:
                        nc.vector.tensor_tensor(out=o2[:], in0=o2[:], in1=b2_sb[:],
                                                op=mybir.AluOpType.add)
                    nc.sync.dma_start(out=out2[b * P : (b + 1) * P, :], in_=o2[:])

    nc.compile()
    return nc


def _install_profile_shim():
    """Provide antenv.axon_hooks (missing on this image) so that
    run_bass_kernel_spmd(trace=True) can capture NTFF profiles."""
    import sys
    import types
    try:
        import antenv.axon_hooks  # noqa: F401
        return
    except ImportError:
        pass
    try:
        from trn_agent_boot.trn_boot import _ntff_profile_via_ctypes
        hook = _ntff_profile_via_ctypes("/opt/axon/libaxon_pjrt.so")
        mod = types.ModuleType("antenv.axon_hooks")
        mod._hook = hook
        mod.get_axon_ntff_profile_hook = lambda: mod._hook
        mod.set_axon_ntff_profile_hook = lambda h: setattr(mod, "_hook", h)
        sys.modules["antenv.axon_hooks"] = mod
    except Exception:
        pass


def kernel(x, edge_index, Wl1, bl1, Wr1, br1, att1, bias1,
           Wl2, bl2, Wr2, br2, att2, bias2, trace=False):
    global LAST_RESULTS
    if trace:
        _install_profile_shim()
    x = np.asarray(x, dtype=np.float32)
    edge_index = np.asarray(edge_index)
    N, F = x.shape
    H1, C1 = np.asarray(att1).shape
    D1 = H1 * C1
    D2 = np.asarray(Wl2).shape[1]

    g1, g2 = 4, 4
    n_pad = ((N + NCORES * P - 1) // (NCORES * P)) * (NCORES * P)
    n_pc = n_pad // NCORES

    meta, m_t = _prep_edges(edge_index, N, n_pc, g1)

    wcat = np.ascontiguousarray(np.concatenate([Wl1, Wr1], axis=1), dtype=np.float32)
    w2cat = np.ascontiguousarray(np.concatenate([Wl2, Wr2], axis=1), dtype=np.float32)
    att_tile = np.tile(np.asarray(att1, np.float32).reshape(1, D1), (P, 1))
    att2_tile = np.tile(np.asarray(att2, np.float32).reshape(1, D2), (P, 1))
    iota_row = np.tile(np.arange(P, dtype=np.float32)[None, :], (P, 1))
    iota_col = np.arange(P, dtype=np.float32)[:, None].copy()
    identity = np.eye(P, dtype=np.float32)

    bias01 = np.concatenate([np.asarray(bl1, np.float32), np.asarray(br1, np.float32)])
    nonzero_bias = bool(np.any(bias01) or np.any(bias1) or np.any(bl2) or np.any(br2) or np.any(bias2))
    consts = dict(wcat=wcat, w2cat=w2cat, att_tile=att_tile, att2_tile=att2_tile,
                  iota_row=iota_row, iota_col=iota_col, identity=identity)
    if nonzero_bias:
        consts["bias1_tile"] = np.tile(np.asarray(bias1, np.float32).reshape(1, D1), (P, 1))
        consts["bias2_tile"] = np.tile(np.asarray(bias2, np.float32).reshape(1, D2), (P, 1))
        consts["ones_row"] = np.ones((1, P), np.float32)
        consts["bias01_row"] = bias01.reshape(1, 2 * D1)
        # fold the ELU "-1" correction into nothing: handled in-kernel already

    nc = _build(n_pad, m_t, g1, g2, consts, nonzero_bias)

    # per-core inputs
    xpadT = np.zeros((P, n_pad), dtype=np.float32)
    xpadT[:, :N] = x.T
    in_maps = []
    for c in range(NCORES):
        in_maps.append({
            "xT": np.ascontiguousarray(xpadT[:, c * n_pc : (c + 1) * n_pc]),
            "meta": meta[c],
            "segf": np.ascontiguousarray(meta[c][:, :, 2]).view(np.float32),
        })

    LAST_RESULTS = run_bass_kernel_spmd(nc, in_maps, core_ids=list(range(NCORES)),
                                        trace=trace)
    out = np.concatenate([r["out2"] for r in LAST_RESULTS.results], axis=0)
    return np.ascontiguousarray(out[:N])
